# revision 1
# baseline (speedup 1.0000x reference)
"""Distributed Trainium2 (Bass/Tile) kernel for a Qwen3-style attention layer.

Full layer: QKV proj -> per-head RMSNorm (q,k) -> RoPE -> GQA SDPA -> o_proj.

Sharding over 8 NeuronCores:
  - tensor-parallel across heads for QKV+attention: core c owns q-heads
    [4c, 4c+4) and kv-head c; hidden_states replicated.
  - AllToAll exchanges attention context so each core ends with all 4096
    context dims for a 256-token slice; o_proj is then token-parallel with
    Wo replicated (streamed). Output: per-core [256, 4096] chunks that the
    host concatenates. No all-reduce needed.

Compute layout: everything lives transposed ([dim, token]) so the PE array
contracts over the partition axis with N=512 moving tiles in bf16.

Schedule: proj(b0) -> attn(b0) || A2A(b0) || proj(b1) -> [hid/weight pools
close] -> attn(b1) with its A2A split in two half-head collectives fired
mid-phase, Wo prefetch running underneath -> o_proj in 2-hid-group blocks,
batch-0 first (covers the tail of the last collective).
"""

import numpy as np
import ml_dtypes

import concourse.bass as bass
import concourse.mybir as mybir
from concourse import bacc
from concourse.tile import TileContext
from concourse.bass_utils import run_bass_kernel_spmd
from concourse.masks import make_identity

F32 = mybir.dt.float32
BF16 = mybir.dt.bfloat16
BF16_NP = ml_dtypes.bfloat16

N_CORES = 8

FULL_CFG = dict(B=2, S=1024, HID=4096, H=32, KV=8, D=128, eps=1e-6)


def build_program(B=2, S=1024, HID=4096, H=32, KV=8, D=128, eps=1e-6):
    cores = N_CORES
    assert D == 128 and H % cores == 0 and KV == cores and B == 2
    HQ = H // cores            # q heads per core
    HH = HQ // 2               # heads per a2a half (last batch)
    T = B * S                  # total tokens
    HCH = HID // 128           # hidden-dim chunks of 128
    TT = min(512, S)           # projection token tile (within batch)
    TPB = S // TT              # projection tiles per batch
    KB = S // 128              # key blocks per batch
    QT = min(512, S)           # attention q tile
    QTB = S // QT              # q tiles per batch
    TC = T // cores            # output tokens per core
    TCB = TC // B              # per-batch token slice per core
    ICH = (H * D) // 128       # o_proj contraction chunks (32)
    OH = min(512, HID // 2)    # o_proj hid tile width
    NHG = HID // OH            # number of hid groups
    GSZ = 2                    # hid groups per o_proj block
    assert NHG % GSZ == 0
    scale = float(D) ** -0.5
    MULT = mybir.AluOpType.mult
    SW = QTB * QT              # full q row per batch (== S)

    nc = bacc.Bacc("TRN2", target_bir_lowering=False, debug=False,
                   num_devices=cores)

    hT = nc.dram_tensor("hT", [B, HCH, 128, S], BF16, kind="ExternalInput")
    wq = nc.dram_tensor("wq", [HQ, 128, HCH * 128], BF16, kind="ExternalInput")
    wk = nc.dram_tensor("wk", [128, HCH * 128], BF16, kind="ExternalInput")
    wv = nc.dram_tensor("wv", [128, HCH * 128], BF16, kind="ExternalInput")
    wo = nc.dram_tensor("wo", [ICH, 128, HID], BF16, kind="ExternalInput")
    cosT = nc.dram_tensor("cosT", [128, S], BF16, kind="ExternalInput")
    csinT = nc.dram_tensor("csinT", [128, S], BF16, kind="ExternalInput")
    qw = nc.dram_tensor("qw", [128, 1], F32, kind="ExternalInput")
    kw = nc.dram_tensor("kw", [128, 1], F32, kind="ExternalInput")
    out = nc.dram_tensor("out", [TC, HID], F32, kind="ExternalOutput")

    with TileContext(nc) as tc:
        with (
            tc.tile_pool(name="const", bufs=1) as cp,
            tc.tile_pool(name="dram", bufs=1, space="DRAM") as dramp,
            tc.tile_pool(name="qkv", bufs=1) as p_qkv,
            tc.tile_pool(name="work", bufs=2) as p_work,
            tc.tile_pool(name="pt", bufs=2) as p_pt,
            tc.tile_pool(name="psum", bufs=1, space="PSUM") as ps_all,
        ):
            ones_s = cp.tile([128, 128], BF16)
            nc.vector.memset(ones_s[:, :], 1.0)
            ident = cp.tile([128, 128], BF16)
            make_identity(nc, ident[:, :])
            eps_s = cp.tile([128, 1], F32)
            nc.vector.memset(eps_s[:, :], eps)
            cos_s = cp.tile([128, S], BF16)
            nc.sync.dma_start(out=cos_s[:, :], in_=cosT[:, :])
            csin_s = cp.tile([128, S], BF16)
            nc.sync.dma_start(out=csin_s[:, :], in_=csinT[:, :])
            qw_s = cp.tile([128, 1], F32)
            nc.sync.dma_start(out=qw_s[:, :], in_=qw[:, :])
            kw_s = cp.tile([128, 1], F32)
            nc.sync.dma_start(out=kw_s[:, :], in_=kw[:, :])

            a2a0_in = dramp.tile([H * D, TCB], BF16, name="a2a0i")
            a2a0_out = dramp.tile([H * D, TCB], BF16, name="a2a0o")
            a2a1_in = [dramp.tile([cores * HH * 128, TCB], BF16,
                                  tag=f"a2a1i{p}", name=f"a2a1i{p}")
                       for p in range(2)]
            a2a1_out = [dramp.tile([cores * HH * 128, TCB], BF16,
                                   tag=f"a2a1o{p}", name=f"a2a1o{p}")
                        for p in range(2)]

            qT_s = p_qkv.tile([128, HQ * T], BF16, tag="qT")
            kT_s = p_qkv.tile([128, T], BF16, tag="kT")
            vnat_s = p_qkv.tile([128, T], BF16, tag="vnat")
            ctxT_s = p_qkv.tile([128, HQ * T], BF16, tag="ctxT")

            def proj(b, p_hid, p_w):
                """QKV projection + norm + rope for batch b."""
                w0_t = p_w.tile([128, HCH * 128], BF16, tag="w", name="w0")
                nc.sync.dma_start(out=w0_t[:, :], in_=wq[0])
                hch = []
                for ch in range(HCH):
                    t_ = p_hid.tile([128, S], BF16, tag="hid", name="hid")
                    nc.sync.dma_start(out=t_[:, :], in_=hT[b, ch, :, :])
                    hch.append(t_)
                for ob in range(HQ + 2):
                    if ob == 0:
                        w_t = w0_t
                    else:
                        w_t = p_w.tile([128, HCH * 128], BF16, tag="w",
                                       name="w")
                        srcw = (wq[ob] if ob < HQ else
                                (wk[:, :] if ob == HQ else wv[:, :]))
                        nc.sync.dma_start(out=w_t[:, :], in_=srcw)
                    for tt in range(TPB):
                        ps = ps_all.tile([128, TT], F32, tag="mm", name="ps", bufs=2, padded_shape=[128, SW])
                        for ch in range(HCH):
                            nc.tensor.matmul(
                                ps[:, :],
                                lhsT=w_t[:, ch * 128:(ch + 1) * 128],
                                rhs=hch[ch][:, tt * TT:(tt + 1) * TT],
                                start=(ch == 0), stop=(ch == HCH - 1))
                        tg = b * S + tt * TT
                        pos = tt * TT
                        if ob <= HQ:
                            is_q = ob < HQ
                            dst = (qT_s[:, ob * T + tg: ob * T + tg + TT]
                                   if is_q else kT_s[:, tg: tg + TT])
                            wcol = qw_s if is_q else kw_s
                            sq = p_work.tile([128, TT], BF16, tag="sq")
                            nc.scalar.square(sq[:, :], ps[:, :])
                            ssq = ps_all.tile([128, TT], F32, tag="aux", name="ssq", bufs=2)
                            nc.tensor.matmul(ssq[:, :], lhsT=ones_s[:, :],
                                             rhs=sq[:, :], start=True,
                                             stop=True)
                            std = p_work.tile([128, TT], F32, tag="std")
                            nc.scalar.activation(
                                std[:, :], ssq[:, :],
                                mybir.ActivationFunctionType.Sqrt,
                                bias=eps_s[:, :], scale=1.0 / D)
                            rs = p_work.tile([128, TT], F32, tag="rs")
                            nc.vector.reciprocal_approx_fast(rs[:, :],
                                                             std[:, :])
                            qn = p_work.tile([128, TT], F32, tag="qn")
                            nc.vector.scalar_tensor_tensor(
                                qn[:, :], in0=ps[:, :], scalar=wcol[:, :],
                                in1=rs[:, :], op0=MULT, op1=MULT)
                            qsw = p_work.tile([128, TT], F32, tag="qsw")
                            nc.sync.dma_start(out=qsw[0:64, :],
                                              in_=qn[64:128, :])
                            nc.sync.dma_start(out=qsw[64:128, :],
                                              in_=qn[0:64, :])
                            t1 = p_work.tile([128, TT], F32, tag="t1")
                            nc.vector.tensor_mul(t1[:, :], qn[:, :],
                                                 cos_s[:, pos: pos + TT])
                            t2 = p_work.tile([128, TT], BF16, tag="t2")
                            nc.vector.tensor_mul(t2[:, :], qsw[:, :],
                                                 csin_s[:, pos: pos + TT])
                            nc.vector.tensor_add(dst, t1[:, :], t2[:, :])
                        else:
                            vt = p_work.tile([128, TT], BF16, tag="vt")
                            nc.scalar.copy(vt[:, :], ps[:, :])
                            for tb in range(TT // 128):
                                vtr = ps_all.tile([128, 128], BF16, tag="aux", name="vtr", bufs=2)
                                nc.tensor.transpose(
                                    vtr[:, :], vt[:, tb * 128:(tb + 1) * 128],
                                    ident[:, :])
                                tbg = tg // 128 + tb
                                nc.scalar.copy(
                                    vnat_s[:, tbg * 128:(tbg + 1) * 128],
                                    vtr[:, :])

            def attn(b, mid_hook=None):
                """Attention for batch b + context shipping (A2A)."""
                last = b == B - 1
                for h in range(HQ):
                    qoff = h * T + b * S
                    pt_t = p_pt.tile([128, KB * SW], BF16, tag="pT",
                                     name="pT")
                    for kb in range(KB):
                        sps = ps_all.tile([128, SW], F32, tag="mm", name="sps", bufs=2)
                        for qt in range(QTB):
                            nc.tensor.matmul(
                                sps[:, qt * QT:(qt + 1) * QT],
                                lhsT=kT_s[:, b * S + kb * 128:
                                          b * S + (kb + 1) * 128],
                                rhs=qT_s[:, qoff + qt * QT:
                                         qoff + (qt + 1) * QT],
                                start=True, stop=True)
                        nc.scalar.activation(
                            pt_t[:, kb * SW:(kb + 1) * SW], sps[:, :],
                            mybir.ActivationFunctionType.Exp, scale=scale)
                    ctxs = [ps_all.tile([128, QT], F32, tag="ctx", name="ctx",
                                     bufs=2) for _ in range(QTB)]
                    for kb in range(KB):
                        tbg = (b * S) // 128 + kb
                        for qt in range(QTB):
                            nc.tensor.matmul(
                                ctxs[qt][:, :],
                                lhsT=vnat_s[:, tbg * 128:(tbg + 1) * 128],
                                rhs=pt_t[:, kb * SW + qt * QT:
                                         kb * SW + (qt + 1) * QT],
                                start=(kb == 0), stop=(kb == KB - 1))
                    denp = p_work.tile([128, SW], BF16, tag="denp")
                    dent = p_work.tile([128, SW], BF16, tag="dent")
                    if KB == 2:
                        nc.vector.tensor_add(denp[:, :], pt_t[:, 0:SW],
                                             pt_t[:, SW:2 * SW])
                    else:
                        assert KB % 4 == 0
                        nc.vector.tensor_add(denp[:, :], pt_t[:, 0:SW],
                                             pt_t[:, SW:2 * SW])
                        nc.vector.tensor_add(dent[:, :],
                                             pt_t[:, 2 * SW:3 * SW],
                                             pt_t[:, 3 * SW:4 * SW])
                        nc.vector.tensor_add(denp[:, :], denp[:, :],
                                             dent[:, :])
                        for g in range(1, KB // 4):
                            nc.vector.tensor_add(
                                dent[:, :],
                                pt_t[:, 4 * g * SW:(4 * g + 1) * SW],
                                pt_t[:, (4 * g + 1) * SW:(4 * g + 2) * SW])
                            nc.vector.tensor_add(denp[:, :], denp[:, :],
                                                 dent[:, :])
                            nc.vector.tensor_add(
                                dent[:, :],
                                pt_t[:, (4 * g + 2) * SW:(4 * g + 3) * SW],
                                pt_t[:, (4 * g + 3) * SW:(4 * g + 4) * SW])
                            nc.vector.tensor_add(denp[:, :], denp[:, :],
                                                 dent[:, :])
                    for qt in range(QTB):
                        dps = ps_all.tile([128, QT], F32, tag="aux", name="dps", bufs=2)
                        nc.tensor.matmul(dps[:, :], lhsT=ones_s[:, :],
                                         rhs=denp[:, qt * QT:(qt + 1) * QT],
                                         start=True, stop=True)
                        rec = p_work.tile([128, QT], F32, tag="rec")
                        nc.vector.reciprocal_approx_fast(rec[:, :], dps[:, :])
                        nc.vector.tensor_mul(
                            ctxT_s[:, qoff + qt * QT: qoff + (qt + 1) * QT],
                            ctxs[qt][:, :], rec[:, :])
                    # ship this head's context
                    if not last:
                        for j in range(cores):
                            nc.sync.dma_start(
                                out=a2a0_in[(j * HQ + h) * 128:
                                            (j * HQ + h + 1) * 128, :],
                                in_=ctxT_s[:, qoff + j * TCB:
                                           qoff + (j + 1) * TCB])
                    else:
                        pi, hh = h // HH, h % HH
                        for j in range(cores):
                            nc.sync.dma_start(
                                out=a2a1_in[pi][(j * HH + hh) * 128:
                                                (j * HH + hh + 1) * 128, :],
                                in_=ctxT_s[:, qoff + j * TCB:
                                           qoff + (j + 1) * TCB])
                        if hh == HH - 1:
                            nc.gpsimd.collective_compute(
                                "AllToAll", mybir.AluOpType.bypass,
                                replica_groups=[list(range(cores))],
                                ins=[a2a1_in[pi].opt()],
                                outs=[a2a1_out[pi].opt()])
                            if pi == 0 and mid_hook is not None:
                                mid_hook()
                if not last:
                    nc.gpsimd.collective_compute(
                        "AllToAll", mybir.AluOpType.bypass,
                        replica_groups=[list(range(cores))],
                        ins=[a2a0_in.opt()],
                        outs=[a2a0_out.opt()])

            # ---- phase 1: proj0, attn0 (+a2a0), proj1 (hid/w pools open) --
            with (
                tc.tile_pool(name="hid", bufs=HCH) as p_hid,
                tc.tile_pool(name="wts", bufs=2) as p_w,
            ):
                sc_ = nc.enter_named_scope("proj0", True)[0]
                proj(0, p_hid, p_w)
                nc.leave_named_scope("proj0", sc_, True)
                sc_ = nc.enter_named_scope("attn0", True)[0]
                attn(0)
                nc.leave_named_scope("attn0", sc_, True)
                sc_ = nc.enter_named_scope("proj1", True)[0]
                proj(1, p_hid, p_w)
                nc.leave_named_scope("proj1", sc_, True)

            # ---- phase 2: attn1 (split a2a) with Wo prefetch underneath --
            with (
                tc.tile_pool(name="wo", bufs=1) as p_wo,
                tc.tile_pool(name="cx", bufs=1) as p_cx,
                tc.tile_pool(name="oo", bufs=4) as p_oo,
            ):
                WOB = ICH + 2

                def load_wo_grp(hgs):
                    wts = {}
                    for ic in range(ICH):
                        wo_t = p_wo.tile([128, GSZ * OH], BF16, tag="wo",
                                         name="wo", bufs=WOB)
                        nc.sync.dma_start(
                            out=wo_t[:, :],
                            in_=wo[ic, :, hgs[0] * OH:
                                   (hgs[0] + GSZ) * OH])
                        for i, hg in enumerate(hgs):
                            wts[(hg, ic)] = wo_t[:, i * OH:(i + 1) * OH]
                    return wts

                # cx0 load can go early (a2a0 long done)
                cx_s = [p_cx.tile([128, ICH * TCB], BF16, tag=f"cx{b}",
                                  name=f"cx{b}") for b in range(B)]
                nc.sync.dma_start(
                    out=cx_s[0][:, :].rearrange("p (ic t) -> p ic t", ic=ICH),
                    in_=a2a0_out[:, :].rearrange("(ic p) t -> p ic t",
                                                 ic=ICH))
                grp0_hgs = [i for i in range(GSZ)]
                grp0_wts = {}

                def _mid_hook():
                    grp0_wts.update(load_wo_grp(grp0_hgs))

                sc_ = nc.enter_named_scope("attn1", True)[0]
                attn(1, mid_hook=_mid_hook)
                nc.leave_named_scope("attn1", sc_, True)

                sc_ = nc.enter_named_scope("oproj", True)[0]
                for grp in range(NHG // GSZ):
                    hgs = [grp * GSZ + i for i in range(GSZ)]
                    wts = grp0_wts if grp == 0 else load_wo_grp(hgs)
                    if grp == 0:
                        # cx1 from the two half-head pieces (after the
                        # first group's weight loads so they aren't
                        # blocked in the DMA queue behind the collective)
                        cxv = cx_s[1][:, :].rearrange(
                            "p (j four t) -> p j four t", four=HQ, t=TCB)
                        for pi in range(2):
                            srcv = a2a1_out[pi][:, :].rearrange(
                                "(j hh p) t -> p hh j t", hh=HH, p=128)
                            for hh in range(HH):
                                nc.sync.dma_start(
                                    out=cxv[:, :, pi * HH + hh, :],
                                    in_=srcv[:, hh])
                    for b in range(B):
                        for hg in hgs:
                            pso = ps_all.tile([TCB, OH], F32, tag="aux",
                                              name="pso", bufs=2)
                            for ic in range(ICH):
                                nc.tensor.matmul(
                                    pso[:, :],
                                    lhsT=cx_s[b][:, ic * TCB:(ic + 1) * TCB],
                                    rhs=wts[(hg, ic)][:, :],
                                    start=(ic == 0), stop=(ic == ICH - 1))
                            ot = p_oo.tile([TCB, OH], F32, tag="oout",
                                           name="oout")
                            nc.vector.tensor_copy(ot[:, :], pso[:, :])
                            nc.sync.dma_start(
                                out=out[b * TCB:(b + 1) * TCB,
                                        hg * OH:(hg + 1) * OH],
                                in_=ot[:, :])
                nc.leave_named_scope("oproj", sc_, True)

    nc.compile()
    return nc


def host_prep(inputs, B=2, S=1024, HID=4096, H=32, KV=8, D=128, eps=1e-6):
    """Shard + lay out the full inputs into per-core in_maps."""
    cores = N_CORES
    HQ = H // cores
    T = B * S
    HCH = HID // 128
    ICH = (H * D) // 128

    hs = np.ascontiguousarray(inputs["hidden_states"], dtype=np.float32)
    fc = np.asarray(inputs["freqs_cis"], dtype=np.float32)
    Wq = np.asarray(inputs["Wq"], dtype=np.float32)
    Wk = np.asarray(inputs["Wk"], dtype=np.float32)
    Wv = np.asarray(inputs["Wv"], dtype=np.float32)
    Wo = np.asarray(inputs["Wo"], dtype=np.float32)
    qnw = np.asarray(inputs["q_norm_w"], dtype=np.float32)
    knw = np.asarray(inputs["k_norm_w"], dtype=np.float32)

    # hidden^T chunks: hT[b, ch, p, s] = hs[b, s, ch*128+p]
    hT = np.ascontiguousarray(
        hs.transpose(0, 2, 1).reshape(B, HCH, 128, S)).astype(BF16_NP)

    cos, sin, nsin = fc[0], fc[1], fc[2]      # [S, D]
    cosT = np.ascontiguousarray(cos.T).astype(BF16_NP)    # [128, S]
    csinT = np.concatenate([nsin.T[0:64], sin.T[64:128]], axis=0)
    csinT = np.ascontiguousarray(csinT).astype(BF16_NP)
    qw_col = np.ascontiguousarray(qnw.reshape(128, 1))
    kw_col = np.ascontiguousarray(knw.reshape(128, 1))

    # Wo^T chunks: wo[ic, p, hid] = Wo[hid, ic*128+p]
    woT = np.ascontiguousarray(Wo.T.reshape(ICH, 128, HID)).astype(BF16_NP)

    def prep_w(Wm, nblocks):
        # [nblocks, p, ch*128] with w[ob, p, ch*128+j] = Wm[ob*128+j, ch*128+p]
        a = Wm.reshape(nblocks, 128, HCH, 128).transpose(0, 3, 2, 1)
        return np.ascontiguousarray(a.reshape(nblocks, 128, HCH * 128)) \
            .astype(BF16_NP)

    in_maps = []
    for c in range(cores):
        Wq_c = Wq[c * HQ * D:(c + 1) * HQ * D]
        Wk_c = Wk[c * D:(c + 1) * D]
        Wv_c = Wv[c * D:(c + 1) * D]
        in_maps.append({
            "hT": hT,
            "wq": prep_w(Wq_c, HQ),
            "wk": prep_w(Wk_c, 1)[0],
            "wv": prep_w(Wv_c, 1)[0],
            "wo": woT,
            "cosT": cosT,
            "csinT": csinT,
            "qw": qw_col,
            "kw": kw_col,
        })
    return in_maps


def gather_output(results, B=2, S=1024, HID=4096, **_):
    cores = N_CORES
    TCB = (B * S) // cores // B
    out = np.empty((B, S, HID), dtype=np.float32)
    for c in range(cores):
        o = results[c]["out"]
        for b in range(B):
            out[b, c * TCB:(c + 1) * TCB] = o[b * TCB:(b + 1) * TCB]
    return out


_NC_CACHE = {}


def kernel(**inputs) -> np.ndarray:
    cfg = FULL_CFG
    key = tuple(sorted(cfg.items()))
    if key not in _NC_CACHE:
        _NC_CACHE[key] = build_program(**cfg)
    nc = _NC_CACHE[key]
    in_maps = host_prep(inputs, **cfg)
    res = run_bass_kernel_spmd(nc, in_maps, core_ids=list(range(N_CORES)))
    return gather_output(res.results, **cfg)



# revision 3
# speedup vs baseline: 1.1228x; 1.1228x over previous
"""Distributed Trainium2 (Bass/Tile) kernel for a Qwen3-style attention layer.

Full layer: QKV proj -> per-head RMSNorm (q,k) -> RoPE -> GQA SDPA -> o_proj.

Sharding over 8 NeuronCores:
  - tensor-parallel across heads for QKV+attention: core c owns q-heads
    [4c, 4c+4) and kv-head c; hidden_states replicated.
  - AllToAll exchanges attention context so each core ends with all 4096
    context dims for a 256-token slice; o_proj is then token-parallel with
    Wo replicated (streamed). Output: per-core [256, 4096] chunks that the
    host concatenates. No all-reduce needed.

Compute layout: everything lives transposed ([dim, token]) so the PE array
contracts over the partition axis with N=512 moving tiles in bf16.

DMA queue discipline (3 independent queues):
  - nc.sync  (SP HWDGE):  hidden/weight loads, rope swaps, a2a ships,
    output stores, first Wo group (interleaved with attn1 heads).
  - nc.scalar (ACT HWDGE): constants + Wo groups 1..3 (streams under
    o_proj while ACT is otherwise idle).
  - nc.gpsimd (SWDGE):     collective triggers + cx context loads.

A2A buffers are laid out [(dst_core*128+p), (head*TCB+t)] so the
post-collective context load is 1KB-contiguous per (partition, source
core) - 8 descriptors per partition instead of a 256B scatter.

Matmuls are paired per stationary operand (proj: ch-outer over both token
tiles; o_proj: ic-outer over both hid groups) - a repeated lhsT matmul
runs ~50ns faster than one that reloads weights.
"""

import numpy as np
import ml_dtypes

import concourse.bass as bass
import concourse.mybir as mybir
from concourse import bacc
from concourse.tile import TileContext
from concourse.bass_utils import run_bass_kernel_spmd
from concourse.masks import make_identity

F32 = mybir.dt.float32
BF16 = mybir.dt.bfloat16
BF16_NP = ml_dtypes.bfloat16

N_CORES = 8

FULL_CFG = dict(B=2, S=1024, HID=4096, H=32, KV=8, D=128, eps=1e-6)


def build_program(B=2, S=1024, HID=4096, H=32, KV=8, D=128, eps=1e-6):
    cores = N_CORES
    assert D == 128 and H % cores == 0 and KV == cores and B == 2
    HQ = H // cores            # q heads per core
    HH = HQ // 2               # heads per a2a half (last batch)
    T = B * S                  # total tokens
    HCH = HID // 128           # hidden-dim chunks of 128
    TT = min(512, S)           # projection token tile (within batch)
    TPB = S // TT              # projection tiles per batch
    KB = S // 128              # key blocks per batch
    QT = min(512, S)           # attention q tile
    QTB = S // QT              # q tiles per batch
    TC = T // cores            # output tokens per core
    TCB = TC // B              # per-batch token slice per core
    ICH = (H * D) // 128       # o_proj contraction chunks (32)
    OH = min(512, HID // 2)    # o_proj hid tile width
    NHG = HID // OH            # number of hid groups
    GSZ = 2                    # hid groups per o_proj block
    assert NHG % GSZ == 0
    scale = float(D) ** -0.5
    MULT = mybir.AluOpType.mult
    SW = QTB * QT              # full q row per batch (== S)

    nc = bacc.Bacc("TRN2", target_bir_lowering=False, debug=False,
                   num_devices=cores)

    hT = nc.dram_tensor("hT", [B, HCH, 128, S], BF16, kind="ExternalInput")
    wq = nc.dram_tensor("wq", [HQ, 128, HCH * 128], BF16, kind="ExternalInput")
    wk = nc.dram_tensor("wk", [128, HCH * 128], BF16, kind="ExternalInput")
    wv = nc.dram_tensor("wv", [128, HCH * 128], BF16, kind="ExternalInput")
    wo = nc.dram_tensor("wo", [ICH, 128, HID], BF16, kind="ExternalInput")
    cosT = nc.dram_tensor("cosT", [128, S], BF16, kind="ExternalInput")
    csinT = nc.dram_tensor("csinT", [128, S], BF16, kind="ExternalInput")
    qw = nc.dram_tensor("qw", [128, 1], F32, kind="ExternalInput")
    kw = nc.dram_tensor("kw", [128, 1], F32, kind="ExternalInput")
    out = nc.dram_tensor("out", [TC, HID], F32, kind="ExternalOutput")

    with TileContext(nc) as tc:
        with (
            tc.tile_pool(name="const", bufs=1) as cp,
            tc.tile_pool(name="dram", bufs=1, space="DRAM") as dramp,
            tc.tile_pool(name="qkv", bufs=1) as p_qkv,
            tc.tile_pool(name="work", bufs=2) as p_work,
            tc.tile_pool(name="pt", bufs=2) as p_pt,
            tc.tile_pool(name="psum", bufs=1, space="PSUM") as ps_all,
        ):
            ones_s = cp.tile([128, 128], BF16)
            nc.vector.memset(ones_s[:, :], 1.0)
            ident = cp.tile([128, 128], BF16)
            make_identity(nc, ident[:, :])
            eps_s = cp.tile([128, 1], F32)
            nc.vector.memset(eps_s[:, :], eps)
            cos_s = cp.tile([128, S], BF16)
            nc.scalar.dma_start(out=cos_s[:, :], in_=cosT[:, :])
            csin_s = cp.tile([128, S], BF16)
            nc.scalar.dma_start(out=csin_s[:, :], in_=csinT[:, :])
            qw_s = cp.tile([128, 1], F32)
            nc.scalar.dma_start(out=qw_s[:, :], in_=qw[:, :])
            kw_s = cp.tile([128, 1], F32)
            nc.scalar.dma_start(out=kw_s[:, :], in_=kw[:, :])

            # A2A buffers: [(dst_core*128 + p), (local_head*TCB + t)] so the
            # received block from src core j sits at rows [j*128, (j+1)*128)
            # with 1KB-contiguous rows.
            a2a0_in = dramp.tile([cores * 128, HQ * TCB], BF16, name="a2a0i")
            a2a0_out = dramp.tile([cores * 128, HQ * TCB], BF16, name="a2a0o")
            a2a1_in = [dramp.tile([cores * 128, HH * TCB], BF16,
                                  tag=f"a2a1i{p}", name=f"a2a1i{p}")
                       for p in range(2)]
            a2a1_out = [dramp.tile([cores * 128, HH * TCB], BF16,
                                   tag=f"a2a1o{p}", name=f"a2a1o{p}")
                        for p in range(2)]

            qT_s = p_qkv.tile([128, HQ * T], BF16, tag="qT")
            kT_s = p_qkv.tile([128, T], BF16, tag="kT")
            vnat_s = p_qkv.tile([128, T], BF16, tag="vnat")
            ctxT_s = p_qkv.tile([128, HQ * T], BF16, tag="ctxT")

            def proj(b, p_hid, p_w):
                """QKV projection + norm + rope for batch b."""
                w0_t = p_w.tile([128, HCH * 128], BF16, tag="w", name="w0")
                nc.sync.dma_start(out=w0_t[:, :], in_=wq[0])
                hch = []
                for ch in range(HCH):
                    t_ = p_hid.tile([128, S], BF16, tag="hid", name="hid")
                    nc.sync.dma_start(out=t_[:, :], in_=hT[b, ch, :, :])
                    hch.append(t_)
                for ob in range(HQ + 2):
                    if ob == 0:
                        w_t = w0_t
                    else:
                        w_t = p_w.tile([128, HCH * 128], BF16, tag="w",
                                       name="w")
                        srcw = (wq[ob] if ob < HQ else
                                (wk[:, :] if ob == HQ else wv[:, :]))
                        nc.sync.dma_start(out=w_t[:, :], in_=srcw)
                    # ch-outer so each weight chunk stays stationary for
                    # both token tiles (paired matmuls).
                    ps = ps_all.tile([128, SW], F32, tag="mm", name="ps",
                                     bufs=2)
                    for ch in range(HCH):
                        lw = w_t[:, ch * 128:(ch + 1) * 128]
                        for tt in range(TPB):
                            nc.tensor.matmul(
                                ps[:, tt * TT:(tt + 1) * TT],
                                lhsT=lw,
                                rhs=hch[ch][:, tt * TT:(tt + 1) * TT],
                                start=(ch == 0), stop=(ch == HCH - 1))
                    for tt in range(TPB):
                        psv = ps[:, tt * TT:(tt + 1) * TT]
                        tg = b * S + tt * TT
                        pos = tt * TT
                        if ob <= HQ:
                            is_q = ob < HQ
                            dst = (qT_s[:, ob * T + tg: ob * T + tg + TT]
                                   if is_q else kT_s[:, tg: tg + TT])
                            wcol = qw_s if is_q else kw_s
                            sq = p_work.tile([128, TT], BF16, tag="sq")
                            nc.scalar.square(sq[:, :], psv)
                            ssq = ps_all.tile([128, TT], F32, tag="aux",
                                              name="ssq", bufs=2)
                            nc.tensor.matmul(ssq[:, :], lhsT=ones_s[:, :],
                                             rhs=sq[:, :], start=True,
                                             stop=True)
                            std = p_work.tile([128, TT], F32, tag="std")
                            nc.scalar.activation(
                                std[:, :], ssq[:, :],
                                mybir.ActivationFunctionType.Sqrt,
                                bias=eps_s[:, :], scale=1.0 / D)
                            rs = p_work.tile([128, TT], F32, tag="rs")
                            nc.vector.reciprocal_approx_fast(rs[:, :],
                                                             std[:, :])
                            qn = p_work.tile([128, TT], F32, tag="qn")
                            nc.vector.scalar_tensor_tensor(
                                qn[:, :], in0=psv, scalar=wcol[:, :],
                                in1=rs[:, :], op0=MULT, op1=MULT)
                            qsw = p_work.tile([128, TT], F32, tag="qsw")
                            nc.sync.dma_start(out=qsw[0:64, :],
                                              in_=qn[64:128, :])
                            nc.sync.dma_start(out=qsw[64:128, :],
                                              in_=qn[0:64, :])
                            t1 = p_work.tile([128, TT], F32, tag="t1")
                            nc.vector.tensor_mul(t1[:, :], qn[:, :],
                                                 cos_s[:, pos: pos + TT])
                            t2 = p_work.tile([128, TT], BF16, tag="t2")
                            nc.vector.tensor_mul(t2[:, :], qsw[:, :],
                                                 csin_s[:, pos: pos + TT])
                            nc.vector.tensor_add(dst, t1[:, :], t2[:, :])
                        else:
                            vt = p_work.tile([128, TT], BF16, tag="vt")
                            nc.scalar.copy(vt[:, :], psv)
                            for tb in range(TT // 128):
                                vtr = ps_all.tile([128, 128], BF16, tag="aux",
                                                  name="vtr", bufs=2)
                                nc.tensor.transpose(
                                    vtr[:, :], vt[:, tb * 128:(tb + 1) * 128],
                                    ident[:, :])
                                tbg = tg // 128 + tb
                                nc.scalar.copy(
                                    vnat_s[:, tbg * 128:(tbg + 1) * 128],
                                    vtr[:, :])

            def attn(b, head_hooks=None):
                """Attention for batch b + context shipping (A2A)."""
                last = b == B - 1
                for h in range(HQ):
                    qoff = h * T + b * S
                    pt_t = p_pt.tile([128, KB * SW], BF16, tag="pT",
                                     name="pT")
                    for kb in range(KB):
                        sps = ps_all.tile([128, SW], F32, tag="mm",
                                          name="sps", bufs=2)
                        for qt in range(QTB):
                            nc.tensor.matmul(
                                sps[:, qt * QT:(qt + 1) * QT],
                                lhsT=kT_s[:, b * S + kb * 128:
                                          b * S + (kb + 1) * 128],
                                rhs=qT_s[:, qoff + qt * QT:
                                         qoff + (qt + 1) * QT],
                                start=True, stop=True)
                        nc.scalar.activation(
                            pt_t[:, kb * SW:(kb + 1) * SW], sps[:, :],
                            mybir.ActivationFunctionType.Exp, scale=scale)
                    ctxs = [ps_all.tile([128, QT], F32, tag="ctx", name="ctx",
                                     bufs=2) for _ in range(QTB)]
                    for kb in range(KB):
                        tbg = (b * S) // 128 + kb
                        for qt in range(QTB):
                            nc.tensor.matmul(
                                ctxs[qt][:, :],
                                lhsT=vnat_s[:, tbg * 128:(tbg + 1) * 128],
                                rhs=pt_t[:, kb * SW + qt * QT:
                                         kb * SW + (qt + 1) * QT],
                                start=(kb == 0), stop=(kb == KB - 1))
                    denp = p_work.tile([128, SW], BF16, tag="denp")
                    dent = p_work.tile([128, SW], BF16, tag="dent")
                    if KB == 2:
                        nc.vector.tensor_add(denp[:, :], pt_t[:, 0:SW],
                                             pt_t[:, SW:2 * SW])
                    else:
                        assert KB % 4 == 0
                        nc.vector.tensor_add(denp[:, :], pt_t[:, 0:SW],
                                             pt_t[:, SW:2 * SW])
                        nc.vector.tensor_add(dent[:, :],
                                             pt_t[:, 2 * SW:3 * SW],
                                             pt_t[:, 3 * SW:4 * SW])
                        nc.vector.tensor_add(denp[:, :], denp[:, :],
                                             dent[:, :])
                        for g in range(1, KB // 4):
                            nc.vector.tensor_add(
                                dent[:, :],
                                pt_t[:, 4 * g * SW:(4 * g + 1) * SW],
                                pt_t[:, (4 * g + 1) * SW:(4 * g + 2) * SW])
                            nc.vector.tensor_add(denp[:, :], denp[:, :],
                                                 dent[:, :])
                            nc.vector.tensor_add(
                                dent[:, :],
                                pt_t[:, (4 * g + 2) * SW:(4 * g + 3) * SW],
                                pt_t[:, (4 * g + 3) * SW:(4 * g + 4) * SW])
                            nc.vector.tensor_add(denp[:, :], denp[:, :],
                                                 dent[:, :])
                    for qt in range(QTB):
                        dps = ps_all.tile([128, QT], F32, tag="aux",
                                          name="dps", bufs=2)
                        nc.tensor.matmul(dps[:, :], lhsT=ones_s[:, :],
                                         rhs=denp[:, qt * QT:(qt + 1) * QT],
                                         start=True, stop=True)
                        rec = p_work.tile([128, QT], F32, tag="rec")
                        nc.vector.reciprocal_approx_fast(rec[:, :], dps[:, :])
                        nc.vector.tensor_mul(
                            ctxT_s[:, qoff + qt * QT: qoff + (qt + 1) * QT],
                            ctxs[qt][:, :], rec[:, :])
                    # ship this head's context
                    if not last:
                        for j in range(cores):
                            nc.sync.dma_start(
                                out=a2a0_in[j * 128:(j + 1) * 128,
                                            h * TCB:(h + 1) * TCB],
                                in_=ctxT_s[:, qoff + j * TCB:
                                           qoff + (j + 1) * TCB])
                    else:
                        pi, hh = h // HH, h % HH
                        for j in range(cores):
                            nc.sync.dma_start(
                                out=a2a1_in[pi][j * 128:(j + 1) * 128,
                                                hh * TCB:(hh + 1) * TCB],
                                in_=ctxT_s[:, qoff + j * TCB:
                                           qoff + (j + 1) * TCB])
                        if hh == HH - 1:
                            nc.gpsimd.collective_compute(
                                "AllToAll", mybir.AluOpType.bypass,
                                replica_groups=[list(range(cores))],
                                ins=[a2a1_in[pi].opt()],
                                outs=[a2a1_out[pi].opt()])
                    if head_hooks is not None:
                        head_hooks(h)
                if not last:
                    nc.gpsimd.collective_compute(
                        "AllToAll", mybir.AluOpType.bypass,
                        replica_groups=[list(range(cores))],
                        ins=[a2a0_in.opt()],
                        outs=[a2a0_out.opt()])

            # ---- phase 1: proj0, attn0 (+a2a0), proj1 (hid/w pools open) --
            with (
                tc.tile_pool(name="hid", bufs=HCH) as p_hid,
                tc.tile_pool(name="wts", bufs=2) as p_w,
            ):
                sc_ = nc.enter_named_scope("proj0", True)[0]
                proj(0, p_hid, p_w)
                nc.leave_named_scope("proj0", sc_, True)
                sc_ = nc.enter_named_scope("attn0", True)[0]
                attn(0)
                nc.leave_named_scope("attn0", sc_, True)
                sc_ = nc.enter_named_scope("proj1", True)[0]
                proj(1, p_hid, p_w)
                nc.leave_named_scope("proj1", sc_, True)

            # ---- phase 2: attn1 (Wo grp0 prefetch interleaved per head),
            # ---- then o_proj streaming Wo grp1..3 on the scalar queue. ----
            with (
                tc.tile_pool(name="wo", bufs=1) as p_wo,
                tc.tile_pool(name="cx", bufs=1) as p_cx,
                tc.tile_pool(name="oo", bufs=4) as p_oo,
            ):
                WOB = ICH + 2

                def load_wo_grp(hgs, ics=None, dma=None):
                    wts = {}
                    if ics is None:
                        ics = range(ICH)
                    if dma is None:
                        dma = nc.scalar.dma_start
                    for ic in ics:
                        wo_t = p_wo.tile([128, GSZ * OH], BF16, tag="wo",
                                         name="wo", bufs=WOB)
                        dma(out=wo_t[:, :],
                            in_=wo[ic, :, hgs[0] * OH:(hgs[0] + GSZ) * OH])
                        for i, hg in enumerate(hgs):
                            wts[(hg, ic)] = wo_t[:, i * OH:(i + 1) * OH]
                    return wts

                # cx0 load can go early (a2a0 long done): one SWDGE DMA,
                # 1KB-contiguous per (partition, src core).
                cx_s = [p_cx.tile([128, ICH * TCB], BF16, tag=f"cx{b}",
                                  name=f"cx{b}") for b in range(B)]
                nc.gpsimd.dma_start(
                    out=cx_s[0][:, :].rearrange("p (j f) -> p j f", j=cores),
                    in_=a2a0_out[:, :].rearrange("(j p) f -> p j f", p=128))

                grp0_hgs = [i for i in range(GSZ)]
                grp0_wts = {}
                ic_per_head = ICH // HQ

                def _head_hook(h):
                    # first Wo group rides the sync queue, 8 tiles per
                    # attn1 head, so descriptor-gen never blocks exp.
                    grp0_wts.update(load_wo_grp(
                        grp0_hgs,
                        ics=range(h * ic_per_head, (h + 1) * ic_per_head),
                        dma=nc.sync.dma_start))

                sc_ = nc.enter_named_scope("attn1", True)[0]
                attn(1, head_hooks=_head_hook)
                nc.leave_named_scope("attn1", sc_, True)

                sc_ = nc.enter_named_scope("oproj", True)[0]
                # cx1 from the two half-collectives (SWDGE queue).
                cxv = cx_s[1][:, :].rearrange("p (j h t) -> p j h t",
                                              j=cores, t=TCB)
                for pi in range(2):
                    nc.gpsimd.dma_start(
                        out=cxv[:, :, pi * HH:(pi + 1) * HH, :],
                        in_=a2a1_out[pi][:, :].rearrange(
                            "(j p) (hh t) -> p j hh t", p=128, t=TCB))

                for grp in range(NHG // GSZ):
                    hgs = [grp * GSZ + i for i in range(GSZ)]
                    wts = grp0_wts if grp == 0 else load_wo_grp(hgs)
                    for b in range(B):
                        psos = [ps_all.tile([TCB, OH], F32, tag="aux",
                                            name="pso", bufs=2)
                                for _ in range(GSZ)]
                        # ic-outer: each cx chunk stays stationary for both
                        # hid groups (paired matmuls).
                        for ic in range(ICH):
                            lw = cx_s[b][:, ic * TCB:(ic + 1) * TCB]
                            for gi in range(GSZ):
                                nc.tensor.matmul(
                                    psos[gi][:, :],
                                    lhsT=lw,
                                    rhs=wts[(hgs[gi], ic)][:, :],
                                    start=(ic == 0), stop=(ic == ICH - 1))
                        for gi, hg in enumerate(hgs):
                            ot = p_oo.tile([TCB, OH], F32, tag="oout",
                                           name="oout")
                            nc.vector.tensor_copy(ot[:, :], psos[gi][:, :])
                            nc.sync.dma_start(
                                out=out[b * TCB:(b + 1) * TCB,
                                        hg * OH:(hg + 1) * OH],
                                in_=ot[:, :])
                nc.leave_named_scope("oproj", sc_, True)

    nc.compile()
    return nc


def host_prep(inputs, B=2, S=1024, HID=4096, H=32, KV=8, D=128, eps=1e-6):
    """Shard + lay out the full inputs into per-core in_maps."""
    cores = N_CORES
    HQ = H // cores
    T = B * S
    HCH = HID // 128
    ICH = (H * D) // 128

    hs = np.ascontiguousarray(inputs["hidden_states"], dtype=np.float32)
    fc = np.asarray(inputs["freqs_cis"], dtype=np.float32)
    Wq = np.asarray(inputs["Wq"], dtype=np.float32)
    Wk = np.asarray(inputs["Wk"], dtype=np.float32)
    Wv = np.asarray(inputs["Wv"], dtype=np.float32)
    Wo = np.asarray(inputs["Wo"], dtype=np.float32)
    qnw = np.asarray(inputs["q_norm_w"], dtype=np.float32)
    knw = np.asarray(inputs["k_norm_w"], dtype=np.float32)

    # hidden^T chunks: hT[b, ch, p, s] = hs[b, s, ch*128+p]
    hT = np.ascontiguousarray(
        hs.transpose(0, 2, 1).reshape(B, HCH, 128, S)).astype(BF16_NP)

    cos, sin, nsin = fc[0], fc[1], fc[2]      # [S, D]
    cosT = np.ascontiguousarray(cos.T).astype(BF16_NP)    # [128, S]
    csinT = np.concatenate([nsin.T[0:64], sin.T[64:128]], axis=0)
    csinT = np.ascontiguousarray(csinT).astype(BF16_NP)
    qw_col = np.ascontiguousarray(qnw.reshape(128, 1))
    kw_col = np.ascontiguousarray(knw.reshape(128, 1))

    # Wo^T chunks: wo[ic, p, hid] = Wo[hid, ic*128+p]
    woT = np.ascontiguousarray(Wo.T.reshape(ICH, 128, HID)).astype(BF16_NP)

    def prep_w(Wm, nblocks):
        # [nblocks, p, ch*128] with w[ob, p, ch*128+j] = Wm[ob*128+j, ch*128+p]
        a = Wm.reshape(nblocks, 128, HCH, 128).transpose(0, 3, 2, 1)
        return np.ascontiguousarray(a.reshape(nblocks, 128, HCH * 128)) \
            .astype(BF16_NP)

    in_maps = []
    for c in range(cores):
        Wq_c = Wq[c * HQ * D:(c + 1) * HQ * D]
        Wk_c = Wk[c * D:(c + 1) * D]
        Wv_c = Wv[c * D:(c + 1) * D]
        in_maps.append({
            "hT": hT,
            "wq": prep_w(Wq_c, HQ),
            "wk": prep_w(Wk_c, 1)[0],
            "wv": prep_w(Wv_c, 1)[0],
            "wo": woT,
            "cosT": cosT,
            "csinT": csinT,
            "qw": qw_col,
            "kw": kw_col,
        })
    return in_maps


def gather_output(results, B=2, S=1024, HID=4096, **_):
    cores = N_CORES
    TCB = (B * S) // cores // B
    out = np.empty((B, S, HID), dtype=np.float32)
    for c in range(cores):
        o = results[c]["out"]
        for b in range(B):
            out[b, c * TCB:(c + 1) * TCB] = o[b * TCB:(b + 1) * TCB]
    return out


_NC_CACHE = {}


def kernel(**inputs) -> np.ndarray:
    cfg = FULL_CFG
    key = tuple(sorted(cfg.items()))
    if key not in _NC_CACHE:
        _NC_CACHE[key] = build_program(**cfg)
    nc = _NC_CACHE[key]
    in_maps = host_prep(inputs, **cfg)
    res = run_bass_kernel_spmd(nc, in_maps, core_ids=list(range(N_CORES)))
    return gather_output(res.results, **cfg)


# revision 8
# speedup vs baseline: 1.1439x; 1.0188x over previous
"""Distributed Trainium2 (Bass/Tile) kernel for a Qwen3-style attention layer.

Full layer: QKV proj -> per-head RMSNorm (q,k) -> RoPE -> GQA SDPA -> o_proj.

Sharding over 8 NeuronCores:
  - tensor-parallel across heads for QKV+attention: core c owns q-heads
    [4c, 4c+4) and kv-head c; hidden_states replicated.
  - AllToAll exchanges attention context so each core ends with all 4096
    context dims for a 256-token slice; o_proj is then token-parallel with
    Wo replicated (streamed). Output: per-core [256, 4096] chunks that the
    host concatenates. No all-reduce needed.

Compute layout: everything lives transposed ([dim, token]) so the PE array
contracts over the partition axis with N=512 moving tiles in bf16.

DMA queue discipline (3 independent queues):
  - nc.sync  (SP HWDGE):  hidden/weight loads, rope swaps, a2a ships,
    output stores, first Wo group (interleaved with attn1 heads).
  - nc.scalar (ACT HWDGE): constants + Wo groups 1..3 (streams under
    o_proj while ACT is otherwise idle).
  - nc.gpsimd (SWDGE):     collective triggers + cx context loads.

A2A buffers are laid out [(dst_core*128+p), (head*TCB+t)] so the
post-collective context load is 1KB-contiguous per (partition, source
core) - 8 descriptors per partition instead of a 256B scatter.

Matmuls are paired per stationary operand (proj: ch-outer over both token
tiles; o_proj: ic-outer over both hid groups) - a repeated lhsT matmul
runs ~50ns faster than one that reloads weights.
"""

import numpy as np
import ml_dtypes

import concourse.bass as bass
import concourse.mybir as mybir
from concourse import bacc
from concourse.tile import TileContext
from concourse.bass_utils import run_bass_kernel_spmd
from concourse.masks import make_identity

F32 = mybir.dt.float32
BF16 = mybir.dt.bfloat16
BF16_NP = ml_dtypes.bfloat16

N_CORES = 8

FULL_CFG = dict(B=2, S=1024, HID=4096, H=32, KV=8, D=128, eps=1e-6)


def build_program(B=2, S=1024, HID=4096, H=32, KV=8, D=128, eps=1e-6):
    cores = N_CORES
    assert D == 128 and H % cores == 0 and KV == cores and B == 2
    HQ = H // cores            # q heads per core
    HH = HQ // 2               # heads per a2a half (last batch)
    T = B * S                  # total tokens
    HCH = HID // 128           # hidden-dim chunks of 128
    TT = min(512, S)           # projection token tile (within batch)
    TPB = S // TT              # projection tiles per batch
    KB = S // 128              # key blocks per batch
    QT = min(512, S)           # attention q tile
    QTB = S // QT              # q tiles per batch
    TC = T // cores            # output tokens per core
    TCB = TC // B              # per-batch token slice per core
    ICH = (H * D) // 128       # o_proj contraction chunks (32)
    OH = min(512, HID // 2)    # o_proj hid tile width
    NHG = HID // OH            # number of hid groups
    GSZ = 2                    # hid groups per o_proj block
    assert NHG % GSZ == 0
    scale = float(D) ** -0.5
    MULT = mybir.AluOpType.mult
    SW = QTB * QT              # full q row per batch (== S)

    nc = bacc.Bacc("TRN2", target_bir_lowering=False, debug=False,
                   num_devices=cores)

    hT = nc.dram_tensor("hT", [B, HCH, 128, S], BF16, kind="ExternalInput")
    wq = nc.dram_tensor("wq", [HQ, 128, HCH * 128], BF16, kind="ExternalInput")
    wk = nc.dram_tensor("wk", [128, HCH * 128], BF16, kind="ExternalInput")
    wv = nc.dram_tensor("wv", [128, HCH * 128], BF16, kind="ExternalInput")
    wo = nc.dram_tensor("wo", [ICH, 128, HID], BF16, kind="ExternalInput")
    cosT = nc.dram_tensor("cosT", [128, S], BF16, kind="ExternalInput")
    csinT = nc.dram_tensor("csinT", [128, S], BF16, kind="ExternalInput")
    qw = nc.dram_tensor("qw", [128, 1], F32, kind="ExternalInput")
    kw = nc.dram_tensor("kw", [128, 1], F32, kind="ExternalInput")
    out = nc.dram_tensor("out", [TC, HID], F32, kind="ExternalOutput")

    with TileContext(nc) as tc:
        with (
            tc.tile_pool(name="const", bufs=1) as cp,
            tc.tile_pool(name="dram", bufs=1, space="DRAM") as dramp,
            tc.tile_pool(name="qkv", bufs=1) as p_qkv,
            tc.tile_pool(name="work", bufs=2) as p_work,
            tc.tile_pool(name="pt", bufs=2) as p_pt,
            tc.tile_pool(name="psum", bufs=1, space="PSUM") as ps_all,
        ):
            ones_s = cp.tile([128, 128], BF16)
            nc.vector.memset(ones_s[:, :], 1.0)
            ident = cp.tile([128, 128], BF16)
            make_identity(nc, ident[:, :])
            # permutation matrix for the rope half-swap (rotate by 64):
            # perm[i, j] = 1 iff j == (i+64) % 128  (self-inverse)
            perm_s = cp.tile([128, 128], BF16)
            nc.vector.memset(perm_s[:, :], 0.0)
            nc.sync.dma_start(out=perm_s[0:64, 64:128], in_=ident[0:64, 0:64])
            nc.sync.dma_start(out=perm_s[64:128, 0:64],
                              in_=ident[64:128, 64:128])
            eps_s = cp.tile([128, 1], F32)
            nc.vector.memset(eps_s[:, :], eps)
            cos_s = cp.tile([128, S], BF16)
            nc.scalar.dma_start(out=cos_s[:, :], in_=cosT[:, :])
            csin_s = cp.tile([128, S], BF16)
            nc.scalar.dma_start(out=csin_s[:, :], in_=csinT[:, :])
            qw_s = cp.tile([128, 1], F32)
            nc.scalar.dma_start(out=qw_s[:, :], in_=qw[:, :])
            kw_s = cp.tile([128, 1], F32)
            nc.scalar.dma_start(out=kw_s[:, :], in_=kw[:, :])

            # Tiny sync collective: absorbs the per-core launch stagger while
            # proj0 computes, so the real collectives find the cores aligned.
            sync_in = dramp.tile([cores, 64], BF16, name="synci")
            sync_out = dramp.tile([cores, 64], BF16, name="synco")
            nc.gpsimd.collective_compute(
                "AllToAll", mybir.AluOpType.bypass,
                replica_groups=[list(range(cores))],
                ins=[sync_in.opt()], outs=[sync_out.opt()])

            # A2A buffers: [(dst_core*128 + p), (local_head*TCB + t)] so the
            # received block from src core j sits at rows [j*128, (j+1)*128)
            # with 1KB-contiguous rows.
            a2a0_in = dramp.tile([cores * 128, HQ * TCB], BF16, name="a2a0i")
            a2a0_out = dramp.tile([cores * 128, HQ * TCB], BF16, name="a2a0o")
            a2a1_in = [dramp.tile([cores * 128, HH * TCB], BF16,
                                  tag=f"a2a1i{p}", name=f"a2a1i{p}")
                       for p in range(2)]
            a2a1_out = [dramp.tile([cores * 128, HH * TCB], BF16,
                                   tag=f"a2a1o{p}", name=f"a2a1o{p}")
                        for p in range(2)]

            qT_s = p_qkv.tile([128, HQ * T], BF16, tag="qT")
            kT_s = p_qkv.tile([128, T], BF16, tag="kT")
            vnat_s = p_qkv.tile([128, T], BF16, tag="vnat")
            ctxT_s = p_qkv.tile([128, HQ * T], BF16, tag="ctxT")

            def proj(b, p_hid, p_w):
                """QKV projection + norm + rope for batch b.

                The rope half-swap runs as a permutation matmul on the PE,
                pipelined one output block behind the projection so the PE
                never waits on the norm chain (and no SBUF-SBUF DMA traffic
                competes with the collectives).
                """
                w0_t = p_w.tile([128, HCH * 128], BF16, tag="w", name="w0")
                for wc in range(4):
                    cs = wc * (HCH // 4) * 128
                    ce = (wc + 1) * (HCH // 4) * 128
                    nc.sync.dma_start(out=w0_t[:, cs:ce], in_=wq[0][:, cs:ce])
                hch = []
                for ch in range(HCH):
                    t_ = p_hid.tile([128, S], BF16, tag="hid", name="hid")
                    nc.sync.dma_start(out=t_[:, :], in_=hT[b, ch, :, :])
                    hch.append(t_)
                pending = []

                def flush_pending():
                    for qn_t, dst, pos in pending:
                        qswp = ps_all.tile([128, TT], F32, tag="aux",
                                           name="qswp", bufs=2)
                        nc.tensor.matmul(qswp[:, :], lhsT=perm_s[:, :],
                                         rhs=qn_t[:, :], start=True,
                                         stop=True)
                        t1 = p_work.tile([128, TT], F32, tag="t1")
                        nc.vector.tensor_mul(t1[:, :], qn_t[:, :],
                                             cos_s[:, pos: pos + TT])
                        t2 = p_work.tile([128, TT], BF16, tag="t2")
                        nc.vector.tensor_mul(t2[:, :], qswp[:, :],
                                             csin_s[:, pos: pos + TT])
                        nc.vector.tensor_add(dst, t1[:, :], t2[:, :])
                    pending.clear()

                for ob in range(HQ + 2):
                    if ob == 0:
                        w_t = w0_t
                    else:
                        w_t = p_w.tile([128, HCH * 128], BF16, tag="w",
                                       name="w")
                        srcw = (wq[ob] if ob < HQ else
                                (wk[:, :] if ob == HQ else wv[:, :]))
                        nc.sync.dma_start(out=w_t[:, :], in_=srcw)
                    # ch-outer so each weight chunk stays stationary for
                    # both token tiles (paired matmuls).
                    ps = ps_all.tile([128, SW], F32, tag="mm", name="ps",
                                     bufs=2)
                    for ch in range(HCH):
                        lw = w_t[:, ch * 128:(ch + 1) * 128]
                        for tt in range(TPB):
                            nc.tensor.matmul(
                                ps[:, tt * TT:(tt + 1) * TT],
                                lhsT=lw,
                                rhs=hch[ch][:, tt * TT:(tt + 1) * TT],
                                start=(ch == 0), stop=(ch == HCH - 1))
                    flush_pending()
                    for tt in range(TPB):
                        psv = ps[:, tt * TT:(tt + 1) * TT]
                        tg = b * S + tt * TT
                        pos = tt * TT
                        if ob <= HQ:
                            is_q = ob < HQ
                            dst = (qT_s[:, ob * T + tg: ob * T + tg + TT]
                                   if is_q else kT_s[:, tg: tg + TT])
                            wcol = qw_s if is_q else kw_s
                            sq = p_work.tile([128, TT], BF16, tag="sq")
                            nc.scalar.square(sq[:, :], psv)
                            ssq = ps_all.tile([128, TT], F32, tag="aux",
                                              name="ssq", bufs=2)
                            nc.tensor.matmul(ssq[:, :], lhsT=ones_s[:, :],
                                             rhs=sq[:, :], start=True,
                                             stop=True)
                            std = p_work.tile([128, TT], F32, tag="std")
                            nc.scalar.activation(
                                std[:, :], ssq[:, :],
                                mybir.ActivationFunctionType.Sqrt,
                                bias=eps_s[:, :], scale=1.0 / D)
                            rs = p_work.tile([128, TT], F32, tag="rs")
                            nc.vector.reciprocal_approx_fast(rs[:, :],
                                                             std[:, :])
                            qn = p_work.tile([128, TT], BF16, tag="qn",
                                             bufs=4)
                            nc.vector.scalar_tensor_tensor(
                                qn[:, :], in0=psv, scalar=wcol[:, :],
                                in1=rs[:, :], op0=MULT, op1=MULT)
                            pending.append((qn, dst, pos))
                        else:
                            vt = p_work.tile([128, TT], BF16, tag="vt")
                            nc.scalar.copy(vt[:, :], psv)
                            for tb in range(TT // 128):
                                vtr = ps_all.tile([128, 128], BF16, tag="aux",
                                                  name="vtr", bufs=2)
                                nc.tensor.transpose(
                                    vtr[:, :], vt[:, tb * 128:(tb + 1) * 128],
                                    ident[:, :])
                                tbg = tg // 128 + tb
                                nc.scalar.copy(
                                    vnat_s[:, tbg * 128:(tbg + 1) * 128],
                                    vtr[:, :])
                flush_pending()

            def attn(b, head_hooks=None):
                """Attention for batch b + context shipping (A2A)."""
                last = b == B - 1
                for h in range(HQ):
                    qoff = h * T + b * S
                    pt_t = p_pt.tile([128, KB * SW], BF16, tag="pT",
                                     name="pT")
                    for kb in range(KB):
                        sps = ps_all.tile([128, SW], F32, tag="mm",
                                          name="sps", bufs=2)
                        for qt in range(QTB):
                            nc.tensor.matmul(
                                sps[:, qt * QT:(qt + 1) * QT],
                                lhsT=kT_s[:, b * S + kb * 128:
                                          b * S + (kb + 1) * 128],
                                rhs=qT_s[:, qoff + qt * QT:
                                         qoff + (qt + 1) * QT],
                                start=True, stop=True)
                        nc.scalar.activation(
                            pt_t[:, kb * SW:(kb + 1) * SW], sps[:, :],
                            mybir.ActivationFunctionType.Exp, scale=scale)
                    ctxs = [ps_all.tile([128, QT], F32, tag="ctx", name="ctx",
                                     bufs=2) for _ in range(QTB)]
                    for kb in range(KB):
                        tbg = (b * S) // 128 + kb
                        for qt in range(QTB):
                            nc.tensor.matmul(
                                ctxs[qt][:, :],
                                lhsT=vnat_s[:, tbg * 128:(tbg + 1) * 128],
                                rhs=pt_t[:, kb * SW + qt * QT:
                                         kb * SW + (qt + 1) * QT],
                                start=(kb == 0), stop=(kb == KB - 1))
                    denp = p_work.tile([128, SW], BF16, tag="denp")
                    dent = p_work.tile([128, SW], BF16, tag="dent")
                    if KB == 2:
                        nc.vector.tensor_add(denp[:, :], pt_t[:, 0:SW],
                                             pt_t[:, SW:2 * SW])
                    else:
                        assert KB % 4 == 0
                        nc.vector.tensor_add(denp[:, :], pt_t[:, 0:SW],
                                             pt_t[:, SW:2 * SW])
                        nc.vector.tensor_add(dent[:, :],
                                             pt_t[:, 2 * SW:3 * SW],
                                             pt_t[:, 3 * SW:4 * SW])
                        nc.vector.tensor_add(denp[:, :], denp[:, :],
                                             dent[:, :])
                        for g in range(1, KB // 4):
                            nc.vector.tensor_add(
                                dent[:, :],
                                pt_t[:, 4 * g * SW:(4 * g + 1) * SW],
                                pt_t[:, (4 * g + 1) * SW:(4 * g + 2) * SW])
                            nc.vector.tensor_add(denp[:, :], denp[:, :],
                                                 dent[:, :])
                            nc.vector.tensor_add(
                                dent[:, :],
                                pt_t[:, (4 * g + 2) * SW:(4 * g + 3) * SW],
                                pt_t[:, (4 * g + 3) * SW:(4 * g + 4) * SW])
                            nc.vector.tensor_add(denp[:, :], denp[:, :],
                                                 dent[:, :])
                    for qt in range(QTB):
                        dps = ps_all.tile([128, QT], F32, tag="aux",
                                          name="dps", bufs=2)
                        nc.tensor.matmul(dps[:, :], lhsT=ones_s[:, :],
                                         rhs=denp[:, qt * QT:(qt + 1) * QT],
                                         start=True, stop=True)
                        rec = p_work.tile([128, QT], F32, tag="rec")
                        nc.vector.reciprocal_approx_fast(rec[:, :], dps[:, :])
                        nc.vector.tensor_mul(
                            ctxT_s[:, qoff + qt * QT: qoff + (qt + 1) * QT],
                            ctxs[qt][:, :], rec[:, :])
                    # ship this head's context
                    if not last:
                        for j in range(cores):
                            nc.sync.dma_start(
                                out=a2a0_in[j * 128:(j + 1) * 128,
                                            h * TCB:(h + 1) * TCB],
                                in_=ctxT_s[:, qoff + j * TCB:
                                           qoff + (j + 1) * TCB])
                    else:
                        pi, hh = h // HH, h % HH
                        for j in range(cores):
                            nc.sync.dma_start(
                                out=a2a1_in[pi][j * 128:(j + 1) * 128,
                                                hh * TCB:(hh + 1) * TCB],
                                in_=ctxT_s[:, qoff + j * TCB:
                                           qoff + (j + 1) * TCB])
                        if hh == HH - 1:
                            nc.gpsimd.collective_compute(
                                "AllToAll", mybir.AluOpType.bypass,
                                replica_groups=[list(range(cores))],
                                ins=[a2a1_in[pi].opt()],
                                outs=[a2a1_out[pi].opt()])
                    if head_hooks is not None:
                        head_hooks(h)
                if not last:
                    nc.gpsimd.collective_compute(
                        "AllToAll", mybir.AluOpType.bypass,
                        replica_groups=[list(range(cores))],
                        ins=[a2a0_in.opt()],
                        outs=[a2a0_out.opt()])

            # ---- phase 1: proj0, attn0 (+a2a0), proj1 (hid/w pools open) --
            with (
                tc.tile_pool(name="hid", bufs=HCH) as p_hid,
                tc.tile_pool(name="wts", bufs=2) as p_w,
            ):
                sc_ = nc.enter_named_scope("proj0", True)[0]
                proj(0, p_hid, p_w)
                nc.leave_named_scope("proj0", sc_, True)
                sc_ = nc.enter_named_scope("attn0", True)[0]
                attn(0)
                nc.leave_named_scope("attn0", sc_, True)
                sc_ = nc.enter_named_scope("proj1", True)[0]
                proj(1, p_hid, p_w)
                nc.leave_named_scope("proj1", sc_, True)

            # ---- phase 2: attn1 (Wo grp0 prefetch interleaved per head),
            # ---- then o_proj streaming Wo grp1..3 on the scalar queue. ----
            with (
                tc.tile_pool(name="wo", bufs=1) as p_wo,
                tc.tile_pool(name="cx", bufs=1) as p_cx,
                tc.tile_pool(name="oo", bufs=6) as p_oo,
            ):
                WOB = ICH + 2

                def load_wo_grp(hgs, ics=None, dma=None):
                    wts = {}
                    if ics is None:
                        ics = range(ICH)
                    if dma is None:
                        dma = nc.scalar.dma_start
                    for ic in ics:
                        wo_t = p_wo.tile([128, GSZ * OH], BF16, tag="wo",
                                         name="wo", bufs=WOB)
                        dma(out=wo_t[:, :],
                            in_=wo[ic, :, hgs[0] * OH:(hgs[0] + GSZ) * OH])
                        for i, hg in enumerate(hgs):
                            wts[(hg, ic)] = wo_t[:, i * OH:(i + 1) * OH]
                    return wts

                # cx0 load can go early (a2a0 long done): one SWDGE DMA,
                # 1KB-contiguous per (partition, src core).
                cx_s = [p_cx.tile([128, ICH * TCB], BF16, tag=f"cx{b}",
                                  name=f"cx{b}") for b in range(B)]
                nc.gpsimd.dma_start(
                    out=cx_s[0][:, :].rearrange("p (j f) -> p j f", j=cores),
                    in_=a2a0_out[:, :].rearrange("(j p) f -> p j f", p=128))

                grp0_hgs = [i for i in range(GSZ)]
                grp0_wts = {}
                ic_per_head = ICH // HQ

                def _head_hook(h):
                    # first Wo group rides the sync queue, 8 tiles per
                    # attn1 head, so descriptor-gen never blocks exp.
                    grp0_wts.update(load_wo_grp(
                        grp0_hgs,
                        ics=range(h * ic_per_head, (h + 1) * ic_per_head),
                        dma=nc.sync.dma_start))

                sc_ = nc.enter_named_scope("attn1", True)[0]
                attn(1, head_hooks=_head_hook)
                nc.leave_named_scope("attn1", sc_, True)

                sc_ = nc.enter_named_scope("oproj", True)[0]
                # cx1 from the two half-collectives. On the sync queue (NOT
                # gpsimd): a load that waits on a collective would stall the
                # Pool FIFO and delay the other half's trigger.
                cxv = cx_s[1][:, :].rearrange("p (j h t) -> p j h t",
                                              j=cores, t=TCB)
                for pi in range(2):
                    nc.sync.dma_start(
                        out=cxv[:, :, pi * HH:(pi + 1) * HH, :],
                        in_=a2a1_out[pi][:, :].rearrange(
                            "(j p) (hh t) -> p j hh t", p=128, t=TCB))

                for grp in range(NHG // GSZ):
                    hgs = [grp * GSZ + i for i in range(GSZ)]
                    wts = grp0_wts if grp == 0 else load_wo_grp(hgs)
                    for b in range(B):
                        psos = [ps_all.tile([TCB, OH], F32, tag="aux",
                                            name="pso", bufs=2)
                                for _ in range(GSZ)]
                        # ic-outer: each cx chunk stays stationary for both
                        # hid groups (paired matmuls).
                        for ic in range(ICH):
                            lw = cx_s[b][:, ic * TCB:(ic + 1) * TCB]
                            for gi in range(GSZ):
                                nc.tensor.matmul(
                                    psos[gi][:, :],
                                    lhsT=lw,
                                    rhs=wts[(hgs[gi], ic)][:, :],
                                    start=(ic == 0), stop=(ic == ICH - 1))
                        for gi, hg in enumerate(hgs):
                            ot = p_oo.tile([TCB, OH], F32, tag="oout",
                                           name="oout")
                            nc.vector.tensor_copy(ot[:, :], psos[gi][:, :])
                            nc.sync.dma_start(
                                out=out[b * TCB:(b + 1) * TCB,
                                        hg * OH:(hg + 1) * OH],
                                in_=ot[:, :])
                nc.leave_named_scope("oproj", sc_, True)

    nc.compile()
    return nc


def host_prep(inputs, B=2, S=1024, HID=4096, H=32, KV=8, D=128, eps=1e-6):
    """Shard + lay out the full inputs into per-core in_maps."""
    cores = N_CORES
    HQ = H // cores
    T = B * S
    HCH = HID // 128
    ICH = (H * D) // 128

    hs = np.ascontiguousarray(inputs["hidden_states"], dtype=np.float32)
    fc = np.asarray(inputs["freqs_cis"], dtype=np.float32)
    Wq = np.asarray(inputs["Wq"], dtype=np.float32)
    Wk = np.asarray(inputs["Wk"], dtype=np.float32)
    Wv = np.asarray(inputs["Wv"], dtype=np.float32)
    Wo = np.asarray(inputs["Wo"], dtype=np.float32)
    qnw = np.asarray(inputs["q_norm_w"], dtype=np.float32)
    knw = np.asarray(inputs["k_norm_w"], dtype=np.float32)

    # hidden^T chunks: hT[b, ch, p, s] = hs[b, s, ch*128+p]
    hT = np.ascontiguousarray(
        hs.transpose(0, 2, 1).reshape(B, HCH, 128, S)).astype(BF16_NP)

    cos, sin, nsin = fc[0], fc[1], fc[2]      # [S, D]
    cosT = np.ascontiguousarray(cos.T).astype(BF16_NP)    # [128, S]
    csinT = np.concatenate([nsin.T[0:64], sin.T[64:128]], axis=0)
    csinT = np.ascontiguousarray(csinT).astype(BF16_NP)
    qw_col = np.ascontiguousarray(qnw.reshape(128, 1))
    kw_col = np.ascontiguousarray(knw.reshape(128, 1))

    # Wo^T chunks: wo[ic, p, hid] = Wo[hid, ic*128+p]
    woT = np.ascontiguousarray(Wo.T.reshape(ICH, 128, HID)).astype(BF16_NP)

    def prep_w(Wm, nblocks):
        # [nblocks, p, ch*128] with w[ob, p, ch*128+j] = Wm[ob*128+j, ch*128+p]
        a = Wm.reshape(nblocks, 128, HCH, 128).transpose(0, 3, 2, 1)
        return np.ascontiguousarray(a.reshape(nblocks, 128, HCH * 128)) \
            .astype(BF16_NP)

    in_maps = []
    for c in range(cores):
        Wq_c = Wq[c * HQ * D:(c + 1) * HQ * D]
        Wk_c = Wk[c * D:(c + 1) * D]
        Wv_c = Wv[c * D:(c + 1) * D]
        in_maps.append({
            "hT": hT,
            "wq": prep_w(Wq_c, HQ),
            "wk": prep_w(Wk_c, 1)[0],
            "wv": prep_w(Wv_c, 1)[0],
            "wo": woT,
            "cosT": cosT,
            "csinT": csinT,
            "qw": qw_col,
            "kw": kw_col,
        })
    return in_maps


def gather_output(results, B=2, S=1024, HID=4096, **_):
    cores = N_CORES
    TCB = (B * S) // cores // B
    out = np.empty((B, S, HID), dtype=np.float32)
    for c in range(cores):
        o = results[c]["out"]
        for b in range(B):
            out[b, c * TCB:(c + 1) * TCB] = o[b * TCB:(b + 1) * TCB]
    return out


_NC_CACHE = {}


def kernel(**inputs) -> np.ndarray:
    cfg = FULL_CFG
    key = tuple(sorted(cfg.items()))
    if key not in _NC_CACHE:
        _NC_CACHE[key] = build_program(**cfg)
    nc = _NC_CACHE[key]
    in_maps = host_prep(inputs, **cfg)
    res = run_bass_kernel_spmd(nc, in_maps, core_ids=list(range(N_CORES)))
    return gather_output(res.results, **cfg)
